# revision 45
# baseline (speedup 1.0000x reference)
"""GCNConv (out = segment_sum(val * (X@W)[col], row)) on 8 TRN2 NeuronCores.

Sharding: output rows (nodes) are sharded across the 8 cores (12500 rows
each); W is replicated.  Each core computes its shard of XW = X @ W, the
shards are AllGathered into a full XW table in every core's DRAM, and each
core then aggregates only its own output rows:

    out[r] = sum over edges (r, c) of  val * XW[c]

The aggregation is implemented as dma_gather of XW rows (the source nodes of
the core's edges, pre-sorted on the host by destination window / source
block) followed by one-hot-matrix matmuls accumulating 128-destination-row
windows in PSUM:  out_win += S @ G  where S[d, e] = val_e * [dest_e == d]
is built on the vector engine from a single fused tensor_scalar
(iota == dest) * val, and G holds the gathered XW rows (one edge per
partition).

Wire format (host <-> device traffic is the dominant cost in this
container -- execution is tunneled over axon RPC at ~40 MB/s
device->host): X^T, W, DEST and VAL travel as fp16, the XW table /
gather / one-hot path runs in fp16 (f32 PSUM accumulation), the gather
index table travels compact as [16, TOTS/16] int16 (replicated to the
128-partition layout the gather ucode needs on device), and OUT travels
as per-row-scaled int8 with the fp16 scales packed into tail rows of the
same tensor (one fetch per core), dequantized on the host.  The device
quantizes with the reciprocal of the fp16-rounded scale, so the scale
rounding cancels in dequant.  End-to-end error vs the f64 reference is
~6.5e-3 relative Frobenius (gate: 2e-2).

execute() keeps a persistent jit executable, device-resident inputs and
reusable (undonated) output-backing zero buffers per Bass program:
repeat calls only re-run the NEFF and pull the ~13 MB quantized output
across the wire (~0.38 s vs ~4.6 s for the naive per-call path).

Host-side preprocessing (inside kernel()) only shards / sorts / pads the
edge list with numpy; all FLOPs and all memory-heavy work run on device.
"""

from contextlib import ExitStack

import numpy as np

import concourse.bass as bass
import concourse.mybir as mybir
from concourse import bacc, tile
from concourse.bass_utils import BassKernelResults, run_bass_kernel_spmd

F32 = mybir.dt.float32
F32R = mybir.dt.float32r
F16 = mybir.dt.float16
I8 = mybir.dt.int8
I16 = mybir.dt.int16
I32 = mybir.dt.int32


class Cfg:
    def __init__(self, n_nodes=100000, in_dim=256, out_dim=128, ncores=8,
                 win=128, grp=4, blk=32768, table_fp16=True, use_f32r=False,
                 wire_fp16=True, out_int8=True):
        # use_f32r: feed fp32 matmul operands as float32r (bitcast view).
        # Plain fp32 matmuls run at 4 cycles/row (two half-speed passes);
        # float32r streams at 1 cycle/row for our [128,128] outputs.
        # wire_fp16: XT/W/DEST/VAL cross the host<->device link as fp16.
        # out_int8: OUT crosses the link as per-row-scaled int8 (the DVE
        # f32->i8 convert rounds-to-nearest-even and saturates; the fp16
        # scales ride in tail rows of the same tensor).  The device->host
        # leg of the axon tunnel runs at ~40 MB/s, so output bytes dominate
        # the warm-call wall clock; int8+scales costs ~6.5e-3 relative
        # error (gate 2e-2).
        self.use_f32r = use_f32r and not table_fp16 and not wire_fp16
        self.out_int8 = out_int8
        self.N = n_nodes
        self.IN = in_dim
        self.OUT = out_dim
        self.P = ncores
        self.R = n_nodes // ncores          # rows (nodes) per core
        self.WIN = win                      # destination window (PSUM partitions)
        self.GRP = grp                      # windows per gather group
        self.BLK = blk                      # gather-table block (int16 index limit)
        self.NW = -(-self.R // win)         # windows per core
        self.NG = -(-self.NW // grp)        # groups per core
        # The XW table is AllGathered in two halves (so block-0/1 gathers can
        # start while the second AllGather is in flight).  Table layout is
        # "half-major": half h holds rows (p, r) for r in [h*R/2, (h+1)*R/2)
        # of every rank p, concatenated by rank.
        self.N2 = self.N // 2               # rows per half
        self.R2 = self.R // 2
        self.NBH = -(-self.N2 // blk)       # blocks per half
        self.NBLK = 2 * self.NBH
        # fp16 XW table: halves gather DMA traffic and enables PE fast
        # weight load + DVE 2-byte perf modes.
        self.table_fp16 = table_fp16
        self.wire_fp16 = wire_fp16
        assert n_nodes % ncores == 0 and self.R % 2 == 0
        assert blk <= 32768

    def remap(self, col):
        """Node id -> position in the half-major AllGather table layout."""
        p, r = np.divmod(col, self.R)
        lo = r < self.R2
        return np.where(lo, p * self.R2 + r,
                        self.N2 + p * self.R2 + (r - self.R2))


CFG = Cfg()


def _plan(cfg, edge_row, edge_col, edge_val):
    """Partition/sort/pad the edge list per core. Returns (static, per_core).

    Static structure (identical for all cores, required for SPMD):
      - SEG/cell_size/cell_off: each (group, block, window) edge segment gets
        a fixed 128-aligned slot range sized to its max count over cores, so
        matmul chunks are window-pure and identically placed on every core
      - instance list: (group, window-in-group, block, chunk) matmul chunks
    Per core:
      - IDX  [16, TOTS//16] int16: gather indices (16-part wrap, compact;
        replicated to [128, .] on device; -1 = skipped tail, 0-pads
        elsewhere are real reads)
      - DEST [128, NINST]: per-chunk-instance local dest row (-1 = inactive)
      - VAL  [128, NINST]: per-chunk-instance edge weight (0 = inactive)
      - NREAL [1, NCELL] i32: live index count per gather call (num_idxs_reg)
    """
    P, R, WIN, GRP, BLK, NBLK = cfg.P, cfg.R, cfg.WIN, cfg.GRP, cfg.BLK, cfg.NBLK
    NW, NG = cfg.NW, cfg.NG
    NCELL = NG * NBLK

    cores = []
    for p in range(P):
        s = np.searchsorted(edge_row, p * R, side="left")
        e = np.searchsorted(edge_row, (p + 1) * R, side="left")
        r = edge_row[s:e].astype(np.int64) - p * R
        c = edge_col[s:e].astype(np.int64)
        v = edge_val[s:e].astype(np.float32)
        w = r // WIN
        g = w // GRP
        pos = cfg.remap(c)                 # position in half-major table
        half = pos // cfg.N2
        off = pos - half * cfg.N2
        b = half * cfg.NBH + off // BLK
        c = off % BLK                      # index within block
        # sort by (group, block, window, col): col-ascending within each
        # window segment gives the gather an ascending HBM address stream
        # (better DRAM bank pipelining) at zero cost.
        order = np.lexsort((c, w, b, g))
        r, c, v, w, g, b = (a[order] for a in (r, c, v, w, g, b))
        cell = g * NBLK + b
        counts = np.bincount(cell, minlength=NCELL)
        cstart = np.concatenate([[0], np.cumsum(counts)[:-1]])
        pos = np.arange(len(r)) - cstart[cell]
        j = w - g * GRP
        cnt_cwj = np.bincount(cell * GRP + j, minlength=NCELL * GRP)
        cnt_cwj = cnt_cwj.reshape(NCELL, GRP)
        cores.append(dict(r=r, c=c, v=v, w=w, g=g, b=b, cell=cell, pos=pos,
                          counts=counts, cnt_cwj=cnt_cwj))

    # Static aligned layout: window segment (cell, j) gets a fixed
    # 128-aligned slot range sized to the max count over cores.  Chunks are
    # then window-pure AND identically placed on every core: no straddle
    # duplicates, no cross-core union slack in the matmul instance list.
    all_cwj = np.stack([cc["cnt_cwj"] for cc in cores])        # [P,NCELL,GRP]
    mx = all_cwj.max(axis=0)                                   # [NCELL,GRP]
    for g in range(NG):
        jmax = min(GRP, NW - g * GRP)
        mx[g * NBLK:(g + 1) * NBLK, jmax:] = 0
    SEG = ((mx + 127) // 128) * 128                            # [NCELL,GRP]
    seg_off = np.concatenate(
        [np.zeros((NCELL, 1), np.int64), np.cumsum(SEG, axis=1)[:, :-1]],
        axis=1)                                                # [NCELL,GRP]
    cell_size = np.maximum(128, SEG.sum(axis=1)).astype(np.int64)  # [NCELL]
    cell_off = np.concatenate([[0], np.cumsum(cell_size)[:-1]]).astype(np.int64)
    TOTS = int(cell_size.sum())

    # instance enumeration (static): for each (g, j): the (b, chunk) matmuls
    inst_list = []
    win_insts = {}
    maxch = int(cell_size.max()) // 128
    L = -np.ones((NCELL, maxch), np.int64)                     # (cell,chunk)->inst
    for g in range(NG):
        jmax = min(GRP, NW - g * GRP)
        for j in range(jmax):
            lst = []
            for b in range(NBLK):
                cell = g * NBLK + b
                if SEG[cell, j] == 0:
                    continue
                ch0 = int(seg_off[cell, j]) // 128
                for ch in range(ch0, ch0 + int(SEG[cell, j]) // 128):
                    inst_id = len(inst_list)
                    inst_list.append((g, j, b, ch))
                    L[cell, ch] = inst_id
                    lst.append((b, ch, inst_id))
            win_insts[(g, j)] = lst
    NINST = len(inst_list)

    # last segment with slots, per cell (for the -1 tail boundary)
    jl = np.where(SEG.any(axis=1), GRP - 1 - np.argmax(SEG[:, ::-1] > 0,
                                                       axis=1), -1)

    per_core = []
    for cc in cores:
        dest = np.full((128, max(NINST, 1)), -1.0, np.float32)
        val = np.zeros((128, max(NINST, 1)), np.float32)
        # -1 = "skip" (no DMA, only legal as a call tail); 0 = real pad read
        idx = np.full(TOTS, -1, np.int16)
        jj = cc["w"] - cc["g"] * GRP
        # rank of each edge within its (cell, window) segment (sorted order
        # is cell-major then window-major, so segments are contiguous runs)
        key = cc["cell"] * GRP + jj
        kcnt = cc["cnt_cwj"].reshape(-1)
        kstart = np.concatenate([[0], np.cumsum(kcnt)[:-1]])
        rank = np.arange(len(key)) - kstart[key]
        local = seg_off[cc["cell"], jj] + rank          # slot within cell
        slot = cell_off[cc["cell"]] + local
        idx[slot] = cc["c"].astype(np.int16)            # block-local index
        # non-negative prefix per cell: everything below the end of this
        # core's last live segment must be a real read (mid-call pads = 0);
        # keep a >=16 floor for the gather ucode's 16-channel index wrap.
        nreal = np.zeros(NCELL, np.int64)
        for cell_id in range(NCELL):
            if jl[cell_id] >= 0:
                bnd = int(seg_off[cell_id, jl[cell_id]]
                          + cc["cnt_cwj"][cell_id, jl[cell_id]])
            else:
                bnd = 0
            bnd = max(bnd, 16)
            base = int(cell_off[cell_id])
            seg = idx[base:base + bnd]
            seg[seg < 0] = 0
            nreal[cell_id] = bnd
        chunk = local // 128
        inst = L[cc["cell"], chunk]
        assert (inst >= 0).all()
        part = local % 128
        dest[part, inst] = (cc["r"] % WIN).astype(np.float32)
        val[part, inst] = cc["v"]
        idx16 = np.ascontiguousarray(idx.reshape(-1, 16).T)    # [16, TOTS//16]
        per_core.append(dict(idx=idx16, dest=dest, val=val,
                             nreal=nreal.astype(np.int32).reshape(1, -1)))

    static = dict(cell_size=cell_size, cell_off=cell_off, TOTS=TOTS,
                  NINST=max(NINST, 1), win_insts=win_insts)
    return static, per_core


def _build(cfg, static, single_core=False, xw_mode="ag", use_gather=True):
    """Trace + schedule + compile the SPMD Bass program (one NEFF, 8 cores).

    single_core=True builds a collective-free variant for TimelineSim cost
    modeling: the gather table is an ExternalInput instead of the AllGather
    output (the AllGather itself costs ~35us extra; see collectives.md).

    xw_mode: "ag" (shard + AllGather), "fill" (no collective; xw_full filled
    with 8 DMA copies of the local shard -- wrong data, crash-bisect only),
    "local" (AllGather with Local instead of Shared scratchpad).
    use_gather=False replaces dma_gather with contiguous DMA reads of the
    same size (wrong data, crash-bisect only).
    """
    R, IN, OUT, WIN, GRP, BLK, NBLK = (cfg.R, cfg.IN, cfg.OUT, cfg.WIN,
                                       cfg.GRP, cfg.BLK, cfg.NBLK)
    NW, NG, N = cfg.NW, cfg.NG, cfg.N
    cell_size, cell_off, TOTS = (static["cell_size"], static["cell_off"],
                                 static["TOTS"])
    NINST, win_insts = static["NINST"], static["win_insts"]

    N2, R2, NBH = cfg.N2, cfg.R2, cfg.NBH
    TDT = F16 if cfg.table_fp16 else F32
    WDT = F16 if cfg.wire_fp16 else F32   # XT / W wire + PE dtype
    ODT = I8 if cfg.out_int8 else (F16 if cfg.wire_fp16 else F32)
    SDT = F16 if cfg.wire_fp16 else F32   # DEST/VAL wire dtype (SBUF keeps f32:
    #                                       is_equal scalars must be float32)
    if cfg.use_f32r:
        def mmc(ap):
            return ap.bitcast(F32R)
    else:
        def mmc(ap):
            return ap

    nc = bacc.Bacc("TRN2", target_bir_lowering=False, debug=False,
                   num_devices=1 if single_core else cfg.P)
    XWFULLd = None
    if single_core:
        XWFULLd = nc.dram_tensor("XWFULL", [N, OUT], TDT,
                                 kind="ExternalInput").ap()
    XTd = nc.dram_tensor("XT", [IN, R], WDT, kind="ExternalInput").ap()
    Wd = nc.dram_tensor("W", [IN, OUT], WDT, kind="ExternalInput").ap()
    IDXd = nc.dram_tensor("IDX", [16, TOTS // 16], I16, kind="ExternalInput").ap()
    DESTd = nc.dram_tensor("DEST", [128, NINST], SDT, kind="ExternalInput").ap()
    VALd = nc.dram_tensor("VAL", [128, NINST], SDT, kind="ExternalInput").ap()
    NCELL = NG * NBLK
    NREALd = nc.dram_tensor("NREAL", [1, NCELL], I32, kind="ExternalInput").ap()
    # int8 mode: the per-row fp16 scales ride in SROWS extra i8 rows at the
    # tail of OUT (one output tensor = one device->host fetch per core;
    # each small-RPC round trip over the tunnel costs ~4 ms).  Scales are
    # rounded to fp16 BEFORE the reciprocal used for quantization, so the
    # fp16 rounding cancels exactly in the host dequant.
    SROWS = (128 * 128 * 2) // OUT if cfg.out_int8 else 0
    if cfg.out_int8:
        assert NW <= 128 and (128 * 128 * 2) % OUT == 0
    OUTd = nc.dram_tensor("OUT", [R + SROWS, OUT], ODT,
                          kind="ExternalOutput").ap()

    blk_rows = [min(BLK, N2 - (b % NBH) * BLK) for b in range(NBLK)]

    with tile.TileContext(nc) as tc:
        with (
            ExitStack() as stack,
            tc.tile_pool(name="dram", bufs=1, space="DRAM") as dram,
            tc.tile_pool(name="consts", bufs=1) as consts,
            tc.tile_pool(name="xtp", bufs=4) as xtp,
            tc.tile_pool(name="xwstage", bufs=3) as xwstage,
            tc.tile_pool(name="gpool", bufs=2) as gpool,
            tc.tile_pool(name="stp", bufs=16) as stp,
            tc.tile_pool(name="outp", bufs=8) as outp,
            tc.tile_pool(name="qp", bufs=8) as qp,
            tc.tile_pool(name="psum_xw", bufs=2, space="PSUM") as psum_xw,
            tc.tile_pool(name="psum_e", bufs=6, space="PSUM") as psum_e,
        ):
            xw_lo0 = dram.tile([R2, OUT], TDT)
            xw_lo1 = dram.tile([R2, OUT], TDT)
            if single_core:
                xw_half = [XWFULLd[0:N2, :], XWFULLd[N2:N, :]]
            else:
                aspace = "Shared" if xw_mode == "ag" else "Local"
                xw_h0 = dram.tile([N2, OUT], TDT, addr_space=aspace)
                xw_h1 = dram.tile([N2, OUT], TDT, addr_space=aspace)
                xw_half = [xw_h0, xw_h1]

            # ---- constants needed immediately (W feeds the first matmul) ----
            w0 = consts.tile([128, OUT], WDT)
            nc.sync.dma_start(w0[:], Wd[0:128, :])
            w1 = consts.tile([128, OUT], WDT)
            nc.sync.dma_start(w1[:], Wd[128:256, :])
            iota_i = consts.tile([128, 128], I32)
            nc.gpsimd.iota(iota_i[:], pattern=[[1, 128]], base=0,
                           channel_multiplier=0)
            iota_f = consts.tile([128, 128], TDT)
            nc.vector.tensor_copy(iota_f[:], iota_i[:])

            # ---- phase 1: xw_local = X_shard @ W  (XT is host-transposed) ----
            PANEL = 1024
            for p0 in range(0, R, PANEL):
                pw = min(PANEL, R - p0)
                xt0 = xtp.tile([128, PANEL], WDT, tag="xt0")
                xt1 = xtp.tile([128, PANEL], WDT, tag="xt1")
                nc.sync.dma_start(xt0[:, :pw], XTd[0:128, p0:p0 + pw])
                nc.sync.dma_start(xt1[:, :pw], XTd[128:256, p0:p0 + pw])
                for t0 in range(0, pw, 128):
                    cnt = min(128, pw - t0)
                    ps = psum_xw.tile([128, OUT], F32)
                    nc.tensor.matmul(ps[:cnt, :], lhsT=mmc(xt0[:, t0:t0 + cnt]),
                                     rhs=mmc(w0[:]), start=True, stop=False)
                    nc.tensor.matmul(ps[:cnt, :], lhsT=mmc(xt1[:, t0:t0 + cnt]),
                                     rhs=mmc(w1[:]), start=False, stop=True)
                    stg = xwstage.tile([128, OUT], TDT)
                    nc.scalar.copy(stg[:cnt, :], ps[:cnt, :])
                    # write to the half-shard tiles (may straddle R2)
                    lo, hi = p0 + t0, p0 + t0 + cnt
                    if lo < R2:
                        c0 = min(hi, R2) - lo
                        nc.sync.dma_start(xw_lo0[lo:lo + c0, :], stg[:c0, :])
                    if hi > R2:
                        s0 = max(lo, R2)
                        nc.sync.dma_start(xw_lo1[s0 - R2:hi - R2, :],
                                          stg[s0 - lo:cnt, :])

            # ---- edge-phase constants: issued AFTER the XT panel DMAs so
            # they don't delay the first XW matmuls on the HWDGE FIFO (they
            # are only consumed once the AllGather completes) ----
            # IDX arrives compact [16, TOTS//16]; the gather ucode needs it
            # replicated in all 8 16-partition groups.
            idx_sb = consts.tile([128, TOTS // 16], I16)
            for k in range(8):
                nc.sync.dma_start(idx_sb[16 * k:16 * (k + 1), :], IDXd[:])
            dest_sb = consts.tile([128, NINST], F32)
            val_sb = consts.tile([128, NINST], F32)
            if SDT is F32:
                nc.sync.dma_start(dest_sb[:], DESTd[:])
                nc.sync.dma_start(val_sb[:], VALd[:])
            else:
                dest_w = consts.tile([128, NINST], SDT)
                nc.sync.dma_start(dest_w[:], DESTd[:])
                nc.vector.tensor_copy(dest_sb[:], dest_w[:])
                val_w = consts.tile([128, NINST], SDT)
                nc.sync.dma_start(val_w[:], VALd[:])
                nc.vector.tensor_copy(val_sb[:], val_w[:])
            nreal_sb = consts.tile([1, NCELL], I32)
            nc.sync.dma_start(nreal_sb[:], NREALd[:])

            # ---- phase 2: AllGather XW shards (two halves, pipelined) ----
            if not single_core:
                for h, (src, dst) in enumerate([(xw_lo0, xw_half[0]),
                                                (xw_lo1, xw_half[1])]):
                    if xw_mode == "fill":
                        for q in range(cfg.P):
                            nc.sync.dma_start(dst[q * R2:(q + 1) * R2, :],
                                              src[:])
                    else:
                        nc.gpsimd.collective_compute(
                            "AllGather", mybir.AluOpType.bypass,
                            replica_groups=[list(range(cfg.P))],
                            ins=[src[:]], outs=[dst[:]],
                        )

            # ---- phase 3: per-group gather + one-hot matmul aggregation ----
            sc_all = None
            if cfg.out_int8:
                sc_all = consts.tile([128, 128], F16)
                nc.vector.memset(sc_all[:], 0.0)
            regs = [stack.enter_context(nc.gpsimd.register(name=f"nreal_r{i}"))
                    for i in range(2)]
            ci = 0
            # per-block max chunks: tiles are allocated at this size so the
            # first-use memset covers the whole pool slot (skipped idx=-1
            # slots must never expose uninitialized SBUF to the matmul)
            nchmax = [max(int(cell_size[g * NBLK + b]) // 128
                          for g in range(NG)) for b in range(NBLK)]
            for g in range(NG):
                gts = []
                for b in range(NBLK):
                    cell = g * NBLK + b
                    nch = int(cell_size[cell]) // 128
                    gt = gpool.tile([128, nchmax[b] * 128], TDT, tag=f"g{b}")
                    off16 = int(cell_off[cell]) // 16
                    if use_gather:
                        if g < 2:
                            nc.vector.memset(gt[:], 0.0)
                        r = regs[ci % 2]
                        ci += 1
                        nc.gpsimd.reg_load(r, nreal_sb[0:1, cell:cell + 1])
                        base = (b % NBH) * BLK
                        nc.gpsimd.dma_gather(
                            gt[:, :nch * 128].rearrange("p (c e) -> p c e",
                                                        e=128),
                            xw_half[b // NBH][base:base + blk_rows[b], :],
                            idx_sb[:, off16:off16 + (nch * 128) // 16],
                            num_idxs=nch * 128,
                            num_idxs_reg=r,
                            elem_size=OUT,
                            single_packet=False,
                        )
                    else:
                        src = xw_half[b // NBH][0:nch * 128, :]
                        nc.sync.dma_start(
                            gt[:, :nch * 128],
                            src.rearrange("(p c) e -> p (c e)", p=128))
                    gts.append(gt)
                jmax = min(GRP, NW - g * GRP)
                for j in range(jmax):
                    w_global = g * GRP + j
                    row0 = w_global * WIN
                    cnt = min(WIN, R - row0)
                    insts = win_insts[(g, j)]
                    ot = outp.tile([128, OUT], ODT)
                    if not insts:
                        nc.vector.memset(ot[:cnt, :], 0.0)
                        if cfg.out_int8:
                            nc.vector.memset(
                                sc_all[:, w_global:w_global + 1], 0.0)
                    else:
                        ps = psum_e.tile([128, OUT], F32)
                        n = len(insts)
                        for k, (b, ch, inst) in enumerate(insts):
                            st = stp.tile([128, 128], TDT)
                            nc.vector.tensor_scalar(
                                out=st[:], in0=iota_f[:],
                                scalar1=dest_sb[:, inst:inst + 1],
                                scalar2=val_sb[:, inst:inst + 1],
                                op0=mybir.AluOpType.is_equal,
                                op1=mybir.AluOpType.mult,
                            )
                            nc.tensor.matmul(
                                ps[:], lhsT=mmc(st[:]),
                                rhs=mmc(gts[b][:, ch * 128:(ch + 1) * 128]),
                                start=(k == 0), stop=(k == n - 1),
                            )
                        if cfg.out_int8:
                            # per-row i8 quantization: scale = max(|row|)/127
                            # (floored at 1e-22), q = round(row / scale); the
                            # f32->i8 convert rounds-to-nearest and saturates
                            mx = qp.tile([128, 1], F32, tag="mx")
                            nc.vector.tensor_reduce(
                                out=mx[:], in_=ps[:],
                                axis=mybir.AxisListType.X,
                                op=mybir.AluOpType.max,
                                apply_absolute_value=True)
                            # floor keeps the fp16 scale normal and the
                            # reciprocal finite for all-zero rows
                            nc.vector.tensor_scalar(
                                out=sc_all[:, w_global:w_global + 1],
                                in0=mx[:], scalar1=1.0 / 127.0, scalar2=1e-4,
                                op0=mybir.AluOpType.mult,
                                op1=mybir.AluOpType.max)
                            rec = qp.tile([128, 1], F32, tag="rec")
                            nc.vector.reciprocal(
                                rec[:], sc_all[:, w_global:w_global + 1])
                            nc.vector.tensor_scalar(
                                out=ot[:], in0=ps[:], scalar1=rec[:],
                                scalar2=None, op0=mybir.AluOpType.mult)
                        else:
                            nc.scalar.copy(ot[:cnt, :], ps[:cnt, :])
                    nc.sync.dma_start(OUTd[row0:row0 + cnt, :], ot[:cnt, :])
            if cfg.out_int8:
                nc.sync.dma_start(
                    OUTd[R:R + SROWS, :].rearrange("(p c) e -> p (c e)",
                                                   p=128),
                    sc_all[:].bitcast(I8))

    nc.compile()
    return nc


def _make_in_maps(cfg, X, W, per_core):
    wdt = np.float16 if cfg.wire_fp16 else np.float32
    sdt = np.float16 if cfg.table_fp16 else np.float32
    X = np.asarray(X, dtype=np.float32)
    W = np.ascontiguousarray(np.asarray(W, dtype=wdt))
    in_maps = []
    for p in range(cfg.P):
        xt = np.ascontiguousarray(X[p * cfg.R:(p + 1) * cfg.R].T.astype(wdt))
        in_maps.append({
            "XT": xt,
            "W": W,
            "IDX": per_core[p]["idx"],
            "DEST": per_core[p]["dest"].astype(sdt),
            "VAL": per_core[p]["val"].astype(sdt),
            "NREAL": per_core[p]["nreal"],
        })
    return in_maps


def prepare(cfg, X, W, edge_row, edge_col, edge_val):
    """Plan + build + compile; returns (nc, in_maps)."""
    edge_row = np.asarray(edge_row)
    edge_col = np.asarray(edge_col)
    edge_val = np.asarray(edge_val)
    if np.any(edge_row[1:] < edge_row[:-1]):   # tolerate unsorted input
        order = np.argsort(edge_row, kind="stable")
        edge_row = edge_row[order]
        edge_col = edge_col[order]
        edge_val = edge_val[order]
    while True:
        static, per_core = _plan(cfg, edge_row, edge_col, edge_val)
        # SBUF budget guard: gather tiles (2 bufs) + idx + dest/val, bytes
        # per partition.  Shrink the window group if a skewed edge
        # distribution would overflow SBUF (uniform-random inputs fit easily).
        tsz = 2 if cfg.table_fp16 else 4
        ssz = 2 if cfg.table_fp16 else 4
        cs = static["cell_size"].reshape(cfg.NG, cfg.NBLK)
        per_part = (2 * int(cs.max(axis=0).sum()) * tsz
                    + static["TOTS"] // 16 * 2 + 2 * static["NINST"] * ssz)
        if per_part <= 140 * 1024 or cfg.GRP == 1:
            break
        cfg = Cfg(cfg.N, cfg.IN, cfg.OUT, cfg.P, cfg.WIN,
                  max(1, cfg.GRP // 2), cfg.BLK, table_fp16=cfg.table_fp16,
                  use_f32r=cfg.use_f32r, wire_fp16=cfg.wire_fp16)
    nc = _build(cfg, static)
    in_maps = _make_in_maps(cfg, X, W, per_core)
    return nc, in_maps


class _CachedRunner:
    """Persistent-jit SPMD executor for one Bass program.

    Mirrors bass2jax.run_bass_via_pjrt's multi-core path, but hoists
    everything reusable out of the per-call path: the jitted executable is
    built once, the (static) external inputs are transferred to the 8
    devices once, and the donated zero output buffers are created on-device
    by a tiny jitted program instead of being shipped from the host.  A
    repeat call therefore costs one NEFF execution plus one device->host
    fetch of the output shards.
    """

    def __init__(self, nc, in_maps, n_cores):
        import jax
        from concourse import bass2jax

        bass2jax.install_neuronx_cc_hook()
        assert not getattr(nc, "dbg_callbacks", None)
        self.nc = nc
        self.in_maps_ref = in_maps
        self.n_cores = n_cores

        partition_name = (nc.partition_id_tensor.name
                          if nc.partition_id_tensor else None)
        in_names, out_names, out_avals, zero_shapes = [], [], [], []
        for alloc in nc.m.functions[0].allocations:
            if not isinstance(alloc, mybir.MemoryLocationSet):
                continue
            assert alloc.memorylocations
            name = alloc.memorylocations[0].name
            if alloc.kind == "ExternalInput":
                if name != partition_name:
                    in_names.append(name)
            elif alloc.kind == "ExternalOutput":
                assert alloc.tensor_shape is not None and alloc.dtype is not None
                out_names.append(name)
                shape = tuple(alloc.tensor_shape)
                dtype = mybir.dt.np(alloc.dtype)
                out_avals.append(jax.core.ShapedArray(shape, dtype))
                zero_shapes.append((shape, dtype))
        if getattr(nc, "dbg_addr", None) is not None:
            in_maps = [{**m, nc.dbg_addr.name: np.zeros((1, 2), np.uint32)}
                       for m in in_maps]
        n_params = len(in_names)
        n_outs = len(out_names)
        in_names = in_names + out_names
        if partition_name is not None:
            in_names = in_names + [partition_name]
        self.out_names = out_names
        self.out_avals = out_avals

        def _body(*args):
            operands = list(args)
            if partition_name is not None:
                operands.append(bass2jax.partition_id_tensor())
            outs = bass2jax._bass_exec_p.bind(
                *operands,
                out_avals=tuple(out_avals),
                in_names=tuple(in_names),
                out_names=tuple(out_names),
                lowering_input_output_aliases=(),
                sim_require_finite=True,
                sim_require_nnan=True,
                nc=nc,
            )
            return tuple(outs)

        from jax.sharding import Mesh, NamedSharding, PartitionSpec
        from jax.experimental.shard_map import shard_map

        devices = jax.devices()[:n_cores]
        assert len(devices) == n_cores
        mesh = Mesh(np.asarray(devices), ("core",))
        self.sharding = NamedSharding(mesh, PartitionSpec("core"))
        in_specs = (PartitionSpec("core"),) * (n_params + n_outs)
        out_specs = (PartitionSpec("core"),) * n_outs
        # The kernel writes every element of every output, so the zero
        # "output-backing" operands don't need donation; keeping them
        # undonated lets one device-resident zero buffer be reused across
        # calls (no per-call zeros transfer or dispatch).
        self._fn = jax.jit(
            shard_map(_body, mesh=mesh, in_specs=in_specs,
                      out_specs=out_specs, check_rep=False),
            keep_unused=True)

        # static inputs: concat per-core along axis 0, transfer once
        concat_in = [
            np.concatenate([np.asarray(in_maps[c][name])
                            for c in range(n_cores)], axis=0)
            for name in in_names[:n_params]
        ]
        self._dev_in = jax.device_put(concat_in, self.sharding)
        # jax.block_until_ready to surface transfer errors here, not on call
        jax.block_until_ready(self._dev_in)

        zero_glob = [((n_cores * s[0],) + tuple(s[1:]), d)
                     for s, d in zero_shapes]
        # One-time transfer of the (undonated, reusable) output-backing
        # zeros; avoids compiling a zeros program on device.
        self._dev_zeros = jax.device_put(
            [np.zeros(s, d) for s, d in zero_glob], self.sharding)
        jax.block_until_ready(self._dev_zeros)

    def __call__(self, per_core_hook=None):
        """Run once; returns per-core dicts of raw host outputs.

        per_core_hook(core, dict) is called as each core's shards land
        (small outputs first), letting postprocessing of core c overlap
        the transfer of core c+1 over the serial tunnel.
        """
        outs = self._fn(*self._dev_in, *self._dev_zeros)
        # small outputs first so hooks never wait on them
        order = sorted(range(len(outs)), key=lambda i: outs[i].nbytes)
        shards = {}
        for i in order:
            sh = sorted(outs[i].addressable_shards,
                        key=lambda s: s.index[0].start or 0)
            assert len(sh) == self.n_cores
            for s in sh:
                s.data.copy_to_host_async()
            shards[i] = sh
        per_core = [{} for _ in range(self.n_cores)]
        for c in range(self.n_cores):
            for i in order:
                per_core[c][self.out_names[i]] = np.asarray(shards[i][c].data)
            if per_core_hook is not None:
                per_core_hook(c, per_core[c])
        return per_core


def _dequant_core(cfg, core, raw, full):
    """One core's raw outputs -> its rows of the full f32 output."""
    o = raw["OUT"]
    dst = full[core * cfg.R:(core + 1) * cfg.R]
    if cfg.out_int8:
        # scales ride in the i8 tail rows: [128, 128] f16, w-major per row
        sc = np.ascontiguousarray(o[cfg.R:]).reshape(-1).view(np.float16)
        scale = (sc.reshape(128, 128)[:, :cfg.NW].astype(np.float32)
                 .T.reshape(-1)[:cfg.R])
        np.multiply(o[:cfg.R], scale[:, None], out=dst, casting="unsafe")
    else:
        dst[:] = o


def _assemble(cfg, results):
    full = np.empty((cfg.P * cfg.R, cfg.OUT), np.float32)
    for p in range(cfg.P):
        _dequant_core(cfg, p, results[p], full)
    return full


def execute(cfg, nc, in_maps, trace=False):
    if trace:
        res = run_bass_kernel_spmd(nc, in_maps, list(range(cfg.P)), trace=True)
        return _assemble(cfg, res.results), res
    runner = getattr(nc, "_gcn_runner", None)
    if runner is None or runner.in_maps_ref is not in_maps:
        runner = _CachedRunner(nc, in_maps, cfg.P)
        nc._gcn_runner = runner
    full = np.empty((cfg.P * cfg.R, cfg.OUT), np.float32)
    results = runner(
        per_core_hook=lambda c, raw: _dequant_core(cfg, c, raw, full))
    res = BassKernelResults(results=results, instructions_and_trace=None,
                            profile_json=None, exec_time_ns=None)
    return full, res


def kernel(X, W, edge_row, edge_col, edge_val):
    nc, in_maps = prepare(CFG, X, W, edge_row, edge_col, edge_val)
    out, _ = execute(CFG, nc, in_maps, trace=False)
    return out


def kernel_traced(X, W, edge_row, edge_col, edge_val, cfg=CFG):
    """kernel() with NTFF profiling when the axon profile hook exists;
    falls back to an untraced run (exec_time_ns None) otherwise."""
    nc, in_maps = prepare(cfg, X, W, edge_row, edge_col, edge_val)
    try:
        return execute(cfg, nc, in_maps, trace=True)
    except (ImportError, ModuleNotFoundError):
        return execute(cfg, nc, in_maps, trace=False)
